# revision 25
# baseline (speedup 1.0000x reference)
"""Dual-path multi-head attention on 8 trn2 NeuronCores.

Sharding: core c = (path p=c//4, batch b=c%4). Each core runs the full
pipeline for one path and one batch element: 3 input projections, 16-head
attention (S=1024, dh=64), output projection. No collectives.

Path 2 cross-wiring (q2 from k; k2,v2 from q) is handled purely by host-side
input routing - every core runs the identical SPMD program.

Precision/speed: all four GEMM groups (Q/K/V projections, output projection)
run as fp8-e4m3 DoubleRow matmuls with hi/lo split operands: x = hi + lo with
both parts fp8 (weights pre-scaled by 32, attention output by 8, so the hi/lo
parts stay in fp8's normal range; the scale is divided back out in the psum
drain). Each K=256 slice takes 3 DoubleRow instructions (hi*hi + hi*lo +
lo*hi, dropping the ~0.1% lo*lo term), i.e. 6 PE cycles per output column vs
bf16's 8 - and accuracy is BETTER than bf16 (hi/lo carries ~11 mantissa
bits). The attention core (scores, exp, PV) stays bf16.

Swapped PV: probs_T blocks are the STATIONARY operand and v1e ([s, e] layout
with a ones column per head, width 65) the moving operand, so each PV matmul
outputs [sq=128, 65] - 65 cycles instead of 1024 - and the softmax
denominator lands in PSUM column 64, a per-partition scalar: normalize is
one DVE tensor_scalar per sq-block, no partition broadcast. The normalized
a1T [sq, e] is transposed back to a1 [e, s] with PE transpose instructions,
then split hi/lo fp8 for the output projection.

Scheduling: DMA transfers are serial, so they are issued in strict
need-order (Q0/K0 inputs first, V-path second, wc last). PV rounds for head
h are emitted in head h+2 (lag-2) so they never wait on exp and the V
projection finishes before the first round. V-proj and Q/K projections are
1-bank psum quarter-chunks woven into head slots as PE filler work.
psum: scores 2x2 banks, proj/transpose 2, pv 2.
"""

import numpy as np
import ml_dtypes

B, S, D, H, DH = 4, 1024, 1024, 16, 64
NB = D // 128  # 8 partition-blocks
HW = 65  # head slot width in v1e (64 data + 1 ones col)
WSC = 32.0  # host-side weight scale before fp8 split
ASC = 8.0   # a1 scale before fp8 split

_compiled = None


def _build():
    import concourse.bass as bass
    import concourse.mybir as mybir
    import concourse.tile as tile
    from concourse import bacc

    dt = mybir.dt
    f32, bf16, f8 = dt.float32, dt.bfloat16, dt.float8e4
    DR = mybir.MatmulPerfMode.DoubleRow

    nc = bacc.Bacc("TRN2", target_bir_lowering=False, debug=False)

    xq_d = nc.dram_tensor("xq", [128, 2, NB, S], f8, kind="ExternalInput")
    xk_d = nc.dram_tensor("xk", [128, 2, NB, S], f8, kind="ExternalInput")
    xv_d = nc.dram_tensor("xv", [128, 2, NB, S], f8, kind="ExternalInput")
    wq_d = nc.dram_tensor("wq", [128, 2, NB, NB, 128], f8, kind="ExternalInput")
    wk_d = nc.dram_tensor("wk", [128, 2, NB, NB, 128], f8, kind="ExternalInput")
    wv_d = nc.dram_tensor("wv", [128, 2, NB, D], f8, kind="ExternalInput")
    wc_d = nc.dram_tensor("wc", [128, 2, NB, NB, 128], f8, kind="ExternalInput")
    bq_d = nc.dram_tensor("bq", [128, NB], f32, kind="ExternalInput")
    bk_d = nc.dram_tensor("bk", [128, NB], f32, kind="ExternalInput")
    bc_d = nc.dram_tensor("bc", [128, NB], f32, kind="ExternalInput")
    bvB_d = nc.dram_tensor("bvB", [128, D], bf16, kind="ExternalInput")
    id_d = nc.dram_tensor("ident", [128, 128], bf16, kind="ExternalInput")
    out_d = nc.dram_tensor("outT", [D, S], f32, kind="ExternalOutput")

    ExpF = mybir.ActivationFunctionType.Exp
    MULT = mybir.AluOpType.mult
    ADD = mybir.AluOpType.add
    SUB = mybir.AluOpType.subtract

    with tile.TileContext(nc) as tc:
        with tc.tile_pool(name="x", bufs=3) as xp, \
             tc.tile_pool(name="wfull", bufs=1) as wfp, \
             tc.tile_pool(name="wblk", bufs=4) as wbp, \
             tc.tile_pool(name="cst", bufs=1) as cp, \
             tc.tile_pool(name="qk", bufs=4) as qkp, \
             tc.tile_pool(name="pers", bufs=1) as prp, \
             tc.tile_pool(name="pt", bufs=3) as ptp, \
             tc.tile_pool(name="a1t", bufs=3) as atp, \
             tc.tile_pool(name="rcs", bufs=4) as rcp, \
             tc.tile_pool(name="ost", bufs=3) as ostp, \
             tc.tile_pool(name="mm", bufs=2, space="PSUM") as mmp, \
             tc.tile_pool(name="vp", bufs=2, space="PSUM") as vpp, \
             tc.tile_pool(name="pv", bufs=2, space="PSUM") as pvp:

            # ---- DMA transfers are serial: issue in strict need-order,
            # hi parts before lo parts so hi*hi matmuls start earliest.
            wqb0 = wbp.tile([128, 2, NB, 128], f8, tag="wblk")
            nc.sync.dma_start(out=wqb0[:, :, :, :], in_=wq_d.ap()[:, :, 0, :, :])
            xq_t = xp.tile([128, 2, NB, S], f8, tag="x")
            nc.scalar.dma_start(out=xq_t[:, 0, 0:4, :], in_=xq_d.ap()[:, 0, 0:4, :])
            bq_t = cp.tile([128, NB], f32)
            nc.gpsimd.dma_start(out=bq_t[:, :], in_=bq_d.ap())
            bk_t = cp.tile([128, NB], f32)
            nc.gpsimd.dma_start(out=bk_t[:, :], in_=bk_d.ap())
            nc.scalar.dma_start(out=xq_t[:, 0, 4:8, :], in_=xq_d.ap()[:, 0, 4:8, :])
            nc.scalar.dma_start(out=xq_t[:, 1, 0:4, :], in_=xq_d.ap()[:, 1, 0:4, :])
            nc.scalar.dma_start(out=xq_t[:, 1, 4:8, :], in_=xq_d.ap()[:, 1, 4:8, :])
            wkb0 = wbp.tile([128, 2, NB, 128], f8, tag="wblk")
            nc.sync.dma_start(out=wkb0[:, :, :, :], in_=wk_d.ap()[:, :, 0, :, :])
            xk_t = xp.tile([128, 2, NB, S], f8, tag="x")
            nc.sync.dma_start(out=xk_t[:, 0, 0:4, :], in_=xk_d.ap()[:, 0, 0:4, :])
            nc.sync.dma_start(out=xk_t[:, 0, 4:8, :], in_=xk_d.ap()[:, 0, 4:8, :])
            nc.sync.dma_start(out=xk_t[:, 1, 0:4, :], in_=xk_d.ap()[:, 1, 0:4, :])
            nc.sync.dma_start(out=xk_t[:, 1, 4:8, :], in_=xk_d.ap()[:, 1, 4:8, :])
            bvB_t = cp.tile([128, D], bf16)
            nc.gpsimd.dma_start(out=bvB_t[:, :], in_=bvB_d.ap())
            wqb1 = wbp.tile([128, 2, NB, 128], f8, tag="wblk")
            nc.gpsimd.dma_start(out=wqb1[:, :, :, :], in_=wq_d.ap()[:, :, 1, :, :])
            wkb1 = wbp.tile([128, 2, NB, 128], f8, tag="wblk")
            nc.scalar.dma_start(out=wkb1[:, :, :, :], in_=wk_d.ap()[:, :, 1, :, :])
            wv_t = wfp.tile([128, 2, NB, D], f8)
            xv_t = xp.tile([128, 2, NB, S], f8, tag="x")
            for l in range(2):
                for i in range(2):
                    nc.sync.dma_start(out=wv_t[:, l, 4 * i:4 * i + 4, :],
                                      in_=wv_d.ap()[:, l, 4 * i:4 * i + 4, :])
                    nc.gpsimd.dma_start(out=xv_t[:, l, 4 * i:4 * i + 4, :],
                                        in_=xv_d.ap()[:, l, 4 * i:4 * i + 4, :])
            id_t = cp.tile([128, 128], bf16)
            nc.gpsimd.dma_start(out=id_t[:, :], in_=id_d.ap())
            bc_t = cp.tile([128, NB], f32)
            nc.gpsimd.dma_start(out=bc_t[:, :], in_=bc_d.ap())

            v1e = prp.tile([128, NB, H * HW], bf16)
            a18 = prp.tile([128, 2, NB, S], f8, tag="a18")
            # partial output-projection accumulators (K-pairs 0-5, bias folded)
            oacc = [prp.tile([128, S], bf16, tag=f"oacc_{m}", name=f"oacc_{m}")
                    for m in range(3)]

            # ones columns of v1e (softmax denominator trick)
            ones_ap = v1e[:, :, :].rearrange("p n (h x) -> p n h x", x=HW)[:, :, :, 64]
            nc.vector.memset(ones_ap, 1.0)

            def dr12(ps, wt, xt, wsl, xsl, start0):
                """12 DoubleRow matmuls: (w_hi+w_lo).T @ (x_hi+x_lo) over the
                full K=1024 (4 K=256 slices), dropping lo*lo. hi*hi first so
                the lo DMAs can trail."""
                for lw, lx in ((0, 0), (0, 1), (1, 0)):
                    for j in range(4):
                        nc.tensor.matmul(
                            ps, wt[:, lw, 2 * j:2 * j + 2, wsl],
                            xt[:, lx, 2 * j:2 * j + 2, xsl],
                            start=(start0 and lw == 0 and lx == 0 and j == 0),
                            stop=(lw == 1 and j == 3),
                            perf_mode=DR,
                        )

            def vproj_q(n2, q):
                """V-proj quarter: out [s-block n2, e-quarter q] -> v1e."""
                ps = vpp.tile([128, 256], f32, tag="vp")
                dr12(ps[:, :], xv_t, wv_t,
                     slice(n2 * 128, (n2 + 1) * 128), slice(q * 256, (q + 1) * 256),
                     True)
                dst = v1e[:, n2, q * 4 * HW:(q + 1) * 4 * HW].rearrange(
                    "p (h x) -> p h x", x=HW)[:, :, 0:64]
                ps_v = ps[:, :].rearrange("p (h x) -> p h x", x=64)
                bv_v = bvB_t[:, q * 256:(q + 1) * 256].rearrange("p (h x) -> p h x", x=64)
                nc.vector.scalar_tensor_tensor(
                    out=dst, in0=ps_v, scalar=1.0 / WSC, in1=bv_v, op0=MULT, op1=ADD)

            def proj_q(wb, x_t, b_t, m, ob, q):
                """Q/K-proj quarter of [e-block m, s] -> bf16 (+ bias)."""
                ps = vpp.tile([128, 256], f32, tag="vp")
                dr12(ps[:, :], wb, x_t,
                     slice(None), slice(q * 256, (q + 1) * 256), True)
                nc.vector.tensor_scalar(
                    ob[:, q * 256:(q + 1) * 256], ps[:, :], 1.0 / WSC,
                    b_t[:, m:m + 1], MULT, ADD)

            def head(h, q1b, k1b, prev, fillers=None):
                """Scores + exp for head h; PV rounds of head h-2 (prev) are
                interleaved so they never wait on this head's exp."""
                po = (h % 2) * 64
                pt = ptp.tile([128, NB, S], bf16, tag="pt")
                for n in range(NB):
                    sps = mmp.tile([128, 2, 512], f32, tag="mm")
                    for c in range(2):
                        nc.tensor.matmul(
                            sps[:, c, :],
                            k1b[po:po + 64, n * 128:(n + 1) * 128],
                            q1b[po:po + 64, c * 512:(c + 1) * 512],
                            start=True, stop=True,
                        )
                    nc.scalar.activation(
                        out=pt[:, n, :].rearrange("p (c s) -> p c s", c=2),
                        in_=sps[:, :, :], func=ExpF, scale=0.125)
                    if prev is not None and n == 0:
                        prev['alloc']()
                    if fillers and n in fillers:
                        for f in fillers[n]:
                            f()
                    if prev is not None and n in (3, 5, 7):
                        prev['round'](n - 3)
                        prev['round'](n - 2)
                if prev is not None:
                    prev['round'](NB - 2)
                    prev['round'](NB - 1)
                    prev['fin']()
                return pt

            def make_pv(h, pt, a1T):
                """Lazy PV + normalize closures for head h (run in head h+2)."""
                po = (h % 2) * 64
                cell = {}

                def alloc():
                    cell['a'] = pvp.tile([128, 4, HW], f32, tag="pv", name=f"pva{h}")
                    cell['b'] = pvp.tile([128, 4, HW], f32, tag="pv", name=f"pvb{h}")

                def rnd(n):
                    # swapped PV: probs_T block stationary, v1e moving.
                    # psum start=True zeroes the whole 2KB bank, so only the
                    # FIRST group per bank sets it (round 0): it zeroes the
                    # bank for all four sibling sq-block groups, which then
                    # accumulate onto clean zeros - no DVE memset needed.
                    for b in range(NB):
                        dst = cell['a'] if b < 4 else cell['b']
                        nc.tensor.matmul(
                            dst[:, b % 4, :],
                            pt[:, n, b * 128:(b + 1) * 128],
                            v1e[:, n, h * HW:(h + 1) * HW],
                            start=(n == 0 and b % 4 == 0),
                            stop=(n == NB - 1),
                            skip_group_check=True,
                        )

                def fin():
                    # normalize: denominator is PSUM column 64 (per-partition)
                    pva, pvb = cell['a'], cell['b']
                    rc = rcp.tile([128, 2, 4], f32, tag="rc")
                    nc.vector.reciprocal(out=rc[:, 0, :], in_=pva[:, :, 64])
                    nc.vector.reciprocal(out=rc[:, 1, :], in_=pvb[:, :, 64])
                    for b in range(NB):
                        src = pva if b < 4 else pvb
                        nc.vector.tensor_scalar(
                            a1T[:, b, po:po + 64], src[:, b % 4, 0:64],
                            rc[:, b // 4, b % 4:b % 4 + 1], None, MULT)

                return {'alloc': alloc, 'round': rnd, 'fin': fin}

            def transposes(m, a1T):
                # a1T [sq, e-pair] -> [e, s] via PE transpose, then hi/lo fp8
                tp = vpp.tile([128, 1024], bf16, tag="vp")
                for b in range(NB):
                    nc.tensor.transpose(
                        tp[:, b * 128:(b + 1) * 128], a1T[:, b, :], id_t[:, :])
                nc.vector.tensor_scalar(
                    a18[:, 0, m, :], tp[:, :], ASC, None, MULT)
                nc.vector.scalar_tensor_tensor(
                    out=a18[:, 1, m, :], in0=tp[:, :], scalar=ASC,
                    in1=a18[:, 0, m, :], op0=MULT, op1=SUB)

            # ---- Q0/K0 chase the x DMAs: emit hi*hi first for quarter
            # pairs, then the lo cross terms, so the PE has hi work to do
            # while the lo pieces are still in flight.
            def proj_q_phased(wb, x_t, b_t, ob, qpair):
                pss = []
                for q in qpair:
                    ps = vpp.tile([128, 256], f32, tag="vp", name=f"p0_{q}")
                    for j in range(4):
                        nc.tensor.matmul(
                            ps[:, :], wb[:, 0, 2 * j:2 * j + 2, :],
                            x_t[:, 0, 2 * j:2 * j + 2, q * 256:(q + 1) * 256],
                            start=(j == 0), stop=False, perf_mode=DR)
                    pss.append(ps)
                for q, ps in zip(qpair, pss):
                    for lw, lx in ((0, 1), (1, 0)):
                        for j in range(4):
                            nc.tensor.matmul(
                                ps[:, :], wb[:, lw, 2 * j:2 * j + 2, :],
                                x_t[:, lx, 2 * j:2 * j + 2, q * 256:(q + 1) * 256],
                                start=False, stop=(lw == 1 and j == 3),
                                perf_mode=DR, skip_group_check=True)
                    nc.vector.tensor_scalar(
                        ob[:, q * 256:(q + 1) * 256], ps[:, :], 1.0 / WSC,
                        b_t[:, 0:1], MULT, ADD)

            q1b = qkp.tile([128, S], bf16, tag="qk")
            k1b = qkp.tile([128, S], bf16, tag="qk")
            proj_q_phased(wqb0, xq_t, bq_t, q1b, (0, 1))
            proj_q_phased(wqb0, xq_t, bq_t, q1b, (2, 3))
            proj_q_phased(wkb0, xk_t, bk_t, k1b, (0, 1))
            proj_q_phased(wkb0, xk_t, bk_t, k1b, (2, 3))

            wc_t = wfp.tile([128, 2, NB, NB, 128], f8, tag="wc")
            state = {('wqb', 1): wqb1, ('wkb', 1): wkb1}

            def load_pair(mm):
                def f():
                    if mm >= 2:
                        state[('wqb', mm)] = wbp.tile(
                            [128, 2, NB, 128], f8, tag="wblk", name=f"wqb{mm}")
                        nc.sync.dma_start(out=state[('wqb', mm)][:, :, :, :],
                                          in_=wq_d.ap()[:, :, mm, :, :])
                        state[('wkb', mm)] = wbp.tile(
                            [128, 2, NB, 128], f8, tag="wblk", name=f"wkb{mm}")
                        nc.scalar.dma_start(out=state[('wkb', mm)][:, :, :, :],
                                            in_=wk_d.ap()[:, :, mm, :, :])
                    state[('q1b', mm)] = qkp.tile([128, S], bf16, tag="qk",
                                                  name=f"q1b{mm}")
                    state[('k1b', mm)] = qkp.tile([128, S], bf16, tag="qk",
                                                  name=f"k1b{mm}")
                return f

            def pcq(mm, q):
                return lambda: proj_q(state[('wqb', mm)], xq_t, bq_t, mm,
                                      state[('q1b', mm)], q)

            def pck(mm, q):
                return lambda: proj_q(state[('wkb', mm)], xk_t, bk_t, mm,
                                      state[('k1b', mm)], q)

            def vpf(n2, q):
                return lambda: vproj_q(n2, q)

            def o_passA(m, q):
                # output-projection partial sum over K-pairs 0-5 (needs only
                # transposes T0-T5): runs as filler in heads 14-15 so the
                # tail only computes the last K-pair
                def f():
                    ps = vpp.tile([128, 256], f32, tag="vp", name=f"oA{m}_{q}")
                    for i, (lw, lx) in enumerate(((0, 0), (0, 1), (1, 0))):
                        for j in range(3):
                            nc.tensor.matmul(
                                ps[:, :], wc_t[:, lw, m, 2 * j:2 * j + 2, :],
                                a18[:, lx, 2 * j:2 * j + 2, q * 256:(q + 1) * 256],
                                start=(i == 0 and j == 0),
                                stop=(i == 2 and j == 2), perf_mode=DR)
                    nc.vector.tensor_scalar(
                        oacc[m][:, q * 256:(q + 1) * 256], ps[:, :],
                        1.0 / (WSC * ASC), bc_t[:, m:m + 1], MULT, ADD)
                return f

            def wc_load():
                def f():
                    nc.sync.dma_start(out=wc_t[:, :, 0:4, :, :],
                                      in_=wc_d.ap()[:, :, 0:4, :, :])
                    nc.scalar.dma_start(out=wc_t[:, :, 4:8, :, :],
                                        in_=wc_d.ap()[:, :, 4:8, :, :])
                return f

            # ---- Q1 in the startup shadow: K0 cannot finish before the
            # xk-lo DMA (~15us), so Q1's quarters (inputs land ~11us) run in
            # the PE's wait for it without delaying the first scores.
            load_pair(1)()
            for q in range(4):
                proj_q(wqb1, xq_t, bq_t, 1, state[('q1b', 1)], q)

            # ---- filler schedule.
            # head 0: K1. heads 1-2: the 32 V-proj quarters, placed after
            # the wv/xv DMAs land (~28us). head 2 also carries Q2; head 3
            # K2; then Q(m+1)/K(m+1) in heads 2m/2m+1 as usual.
            fill = {h: {} for h in range(H)}
            fill[0] = {0: [pck(1, 0)], 2: [pck(1, 1)], 4: [pck(1, 2)],
                       6: [pck(1, 3)]}
            fill[1] = {0: [load_pair(2)],
                       1: [vpf(0, 0), vpf(0, 1)],
                       3: [vpf(0, 2), vpf(0, 3), vpf(1, 0), vpf(1, 1)],
                       5: [vpf(1, 2), vpf(1, 3), vpf(2, 0), vpf(2, 1)],
                       7: [vpf(2, 2), vpf(2, 3), vpf(3, 0), vpf(3, 1)]}
            fill[2] = {0: [pcq(2, 0)],
                       1: [vpf(3, 2), vpf(3, 3), vpf(4, 0), vpf(4, 1)],
                       2: [pcq(2, 1)],
                       3: [vpf(4, 2), vpf(4, 3), vpf(5, 0), vpf(5, 1)],
                       4: [pcq(2, 2)],
                       5: [vpf(5, 2), vpf(5, 3), vpf(6, 0), vpf(6, 1)],
                       6: [pcq(2, 3)],
                       7: [vpf(6, 2), vpf(6, 3), vpf(7, 0), vpf(7, 1),
                           vpf(7, 2), vpf(7, 3)]}
            fill[3] = {0: [pck(2, 0)], 2: [pck(2, 1)], 4: [pck(2, 2)],
                       6: [pck(2, 3)]}
            for m in range(2, NB - 1):
                fill[2 * m][0] = [load_pair(m + 1), pcq(m + 1, 0)]
                fill[2 * m][2] = [pcq(m + 1, 1)]
                fill[2 * m][4] = [pcq(m + 1, 2)]
                fill[2 * m][6] = [pcq(m + 1, 3)]
                fill[2 * m + 1][0] = [pck(m + 1, 0)]
                fill[2 * m + 1][2] = [pck(m + 1, 1)]
                fill[2 * m + 1][4] = [pck(m + 1, 2)]
                fill[2 * m + 1][6] = [pck(m + 1, 3)]
            fill[8].setdefault(5, []).append(wc_load())
            fill[14][3] = [o_passA(0, 0), o_passA(0, 1)]
            fill[14][5] = [o_passA(0, 2), o_passA(0, 3)]
            fill[14][7] = [o_passA(1, 0), o_passA(1, 1)]
            fill[15][1] = [o_passA(1, 2), o_passA(1, 3)]
            fill[15][3] = [o_passA(2, 0), o_passA(2, 1)]
            fill[15][5] = [o_passA(2, 2), o_passA(2, 3)]

            from collections import deque
            pending = deque()
            a1T = None
            a1Ts = {}
            for h in range(H):
                m = h // 2
                if h % 2 == 0:
                    a1T = atp.tile([128, NB, 128], bf16, tag="a1T", name=f"a1T{m}")
                    a1Ts[m] = a1T
                    # transpose pair m-2 (normalize finished in head 2m-1)
                    if m >= 2:
                        mm_ = m - 2
                        fill[h].setdefault(1, []).append(
                            (lambda m2, t2: lambda: transposes(m2, t2))(
                                mm_, a1Ts[mm_]))
                prev = pending.popleft() if h >= 2 else None
                pt = head(h, q1b, k1b, prev, fillers=fill[h])
                pending.append(make_pv(h, pt, a1T))
                if h % 2 == 1 and m < NB - 1:
                    q1b, k1b = state[('q1b', m + 1)], state[('k1b', m + 1)]

            # tail: heads 14/15 PV + normalize + last two transposes
            for i, pv in enumerate(pending):
                pv['alloc']()
                for r in range(NB):
                    pv['round'](r)
                pv['fin']()
                transposes(NB - 2 + i, a1Ts[NB - 2 + i])

            # ---- output projection (fp8 hi/lo DoubleRow quarters) ----
            for m in range(NB):
                ot = ostp.tile([128, 1024], f32, tag="ost")
                for q in range(4):
                    ps = vpp.tile([128, 256], f32, tag="vp")
                    if m < 3:
                        # K-pairs 0-5 precomputed in oacc; add the last pair
                        for i, (lw, lx) in enumerate(((0, 0), (0, 1), (1, 0))):
                            nc.tensor.matmul(
                                ps[:, :], wc_t[:, lw, m, 6:8, :],
                                a18[:, lx, 6:8, q * 256:(q + 1) * 256],
                                start=(i == 0), stop=(i == 2), perf_mode=DR)
                        nc.vector.scalar_tensor_tensor(
                            out=ot[:, q * 256:(q + 1) * 256], in0=ps[:, :],
                            scalar=1.0 / (WSC * ASC),
                            in1=oacc[m][:, q * 256:(q + 1) * 256],
                            op0=MULT, op1=ADD)
                        continue
                    dr12(ps[:, :], wc_t[:, :, m, :, :],
                         a18, slice(None), slice(q * 256, (q + 1) * 256), True)
                    nc.vector.tensor_scalar(
                        ot[:, q * 256:(q + 1) * 256], ps[:, :], 1.0 / (WSC * ASC),
                        bc_t[:, m:m + 1], MULT, ADD)
                    if m == NB - 1:
                        # last block: store per-quarter so the final DMA
                        # chain pipelines with the drains
                        nc.sync.dma_start(
                            out=out_d.ap()[m * 128:(m + 1) * 128,
                                           q * 256:(q + 1) * 256],
                            in_=ot[:, q * 256:(q + 1) * 256])
                if m < NB - 1:
                    nc.sync.dma_start(
                        out=out_d.ap()[m * 128:(m + 1) * 128, :],
                        in_=ot[:, :])

    nc.compile()
    return nc


def _get_nc():
    global _compiled
    if _compiled is None:
        _compiled = _build()
    return _compiled


_E4 = ml_dtypes.float8_e4m3


def _hilo(a):
    """f32 array -> fp8 hi/lo stacked on a new leading axis."""
    hi = a.astype(_E4)
    lo = (a - hi.astype(np.float32)).astype(_E4)
    return hi, lo


def _make_in_maps(q, k, v, Wq, bq, Wk, bk, Wv, bv, Wq2, bq2, Wk2, bk2, Wv2, bv2,
                  Wc, bc, Wc2, bc2):
    bf16 = ml_dtypes.bfloat16

    def xpack(x):  # [s, d] -> [p, 2, n, s] fp8 hi/lo
        x = np.asarray(x, np.float32)
        base = np.ascontiguousarray(x.reshape(S, NB, 128).transpose(2, 1, 0))
        hi, lo = _hilo(base)
        return np.ascontiguousarray(
            np.stack([hi, lo], axis=0).transpose(1, 0, 2, 3))

    def wpack(w):  # W[e, d] -> [p, 2, m, n, e'] fp8 hi/lo of 32*W
        w = np.asarray(w, np.float32) * WSC
        base = np.ascontiguousarray(
            w.reshape(NB, 128, NB, 128).transpose(3, 0, 2, 1))
        hi, lo = _hilo(base)
        return np.ascontiguousarray(
            np.stack([hi, lo], axis=0).transpose(1, 0, 2, 3, 4))

    def wvpack(w):  # Wv[e, d] -> [p, 2, n, e] fp8 hi/lo of 32*Wv
        w = np.asarray(w, np.float32) * WSC
        base = np.ascontiguousarray(w.T.reshape(NB, 128, D).transpose(1, 0, 2))
        hi, lo = _hilo(base)
        return np.ascontiguousarray(
            np.stack([hi, lo], axis=0).transpose(1, 0, 2, 3))

    def btile(b):
        return np.ascontiguousarray(np.asarray(b, np.float32).reshape(NB, 128).T)

    def brep(b):
        return np.ascontiguousarray(
            np.broadcast_to(np.asarray(b, np.float32), (128, D))).astype(bf16)

    ident = np.ascontiguousarray(np.eye(128, dtype=np.float32)).astype(bf16)

    paths = [
        dict(wq=wpack(Wq), wk=wpack(Wk), wv=wvpack(Wv), wc=wpack(Wc),
             bq=btile(bq), bk=btile(bk), bc=btile(bc), bvB=brep(bv), ident=ident),
        dict(wq=wpack(Wq2), wk=wpack(Wk2), wv=wvpack(Wv2), wc=wpack(Wc2),
             bq=btile(bq2), bk=btile(bk2), bc=btile(bc2), bvB=brep(bv2), ident=ident),
    ]
    in_maps = []
    xq_b = [xpack(np.asarray(q)[b]) for b in range(4)]
    xk_b = [xpack(np.asarray(k)[b]) for b in range(4)]
    xv_b = [xpack(np.asarray(v)[b]) for b in range(4)]
    for c in range(8):
        p, b = c // 4, c % 4
        if p == 0:
            xq, xk, xv = xq_b[b], xk_b[b], xv_b[b]
        else:
            # path 2: q2 from k; k2, v2 from q
            xq, xk, xv = xk_b[b], xq_b[b], xq_b[b]
        in_maps.append(dict(paths[p], xq=xq, xk=xk, xv=xv))
    return in_maps


def _run(in_maps, trace=False):
    from concourse.bass_utils import run_bass_kernel_spmd
    nc = _get_nc()
    return run_bass_kernel_spmd(nc, in_maps, core_ids=list(range(8)), trace=trace)


def kernel(**inputs):
    in_maps = _make_in_maps(**inputs)
    try:
        res = _run(in_maps)
    except Exception:
        # transient NRT_EXEC_UNIT_UNRECOVERABLE has been observed when a
        # prior process crashed mid-execution; one retry reloads the NEFF
        res = _run(in_maps)
    out1 = np.stack([res.results[b]["outT"].T for b in range(4)]).astype(np.float32)
    out2 = np.stack([res.results[4 + b]["outT"].T for b in range(4)]).astype(np.float32)
    return out1, out2
